# revision 3
# baseline (speedup 1.0000x reference)
"""Trainium2 Bass kernel for nn_FCAutoEncoder (ragged_sequence).

Strategy:
  * Host: bucket rows by seq_length (5 sizes), split each bucket evenly
    across 8 cores (pure data parallel), transpose to feature-major
    [1008, R] per core so activations live as [feat_part, batch_free].
  * Device (per core, identical SPMD program): per bucket k, per column
    chunk (<=512): expand with Win[k] restricted to its true s_k input
    features, shared 1008-512-256-128-256-512-1008 MLP, contract with
    Wout[k] restricted to s_k output features.  All matmuls run as
    float32r (full-rate ~TF32 path, fp32 PSUM accumulation).  PSUM is
    evacuated with fused bias(+ReLU) on ScalarE/VectorE.
  * Host: transpose back, scatter rows to original order; rows beyond
    s_k and rows with unknown lengths are zero.
"""
import math
import os
import sys

sys.path.insert(0, "/opt/trn_rl_repo")

import numpy as np

SIZES = (36, 72, 144, 288, 1008)
BASE = 1008
H1, H2, LAT = 512, 256, 128
N_CORES = 8
MAX_CHUNK = 512
ACT_BUFS = 20

_last_exec_ns = None
_prog_cache = {}


def _tiles(n, t=128):
    return [(s, min(t, n - s)) for s in range(0, n, t)]


def _chunks(c, maxn=MAX_CHUNK):
    """Split c (even) into even-sized chunks <= maxn.

    float32r matmuls require an even moving dim, so every chunk is even.
    """
    if c <= 0:
        return []
    assert c % 2 == 0
    half = c // 2
    n = (c + maxn - 1) // maxn
    base, rem = divmod(half, n)
    out, off = [], 0
    for i in range(n):
        sz = 2 * (base + (1 if i < rem else 0))
        out.append((off, sz))
        off += sz
    return out


def _bias_layout():
    """Fixed column order of the packed [128, NB] bias tensor."""
    cols = []
    for k in range(5):
        for (ms, mp) in _tiles(BASE):
            cols.append(("exp", k, ms, mp))
    for (js, jp) in _tiles(H1):
        cols.append(("L1", 0, js, jp))
    for (js, jp) in _tiles(H2):
        cols.append(("L2", 0, js, jp))
    for (js, jp) in _tiles(LAT):
        cols.append(("L3", 0, js, jp))
    for (js, jp) in _tiles(H2):
        cols.append(("D1", 0, js, jp))
    for (js, jp) in _tiles(H1):
        cols.append(("D2", 0, js, jp))
    for (ms, mp) in _tiles(BASE):
        cols.append(("D3", 0, ms, mp))
    for k in range(5):
        for (os_, op) in _tiles(SIZES[k]):
            cols.append(("out", k, os_, op))
    return cols


def _build_program(c_ks, R):
    import concourse.bacc as bacc
    import concourse.mybir as mybir
    from concourse import tile

    f32 = mybir.dt.float32
    f32r = mybir.dt.float32r
    AF = mybir.ActivationFunctionType
    ALU = mybir.AluOpType

    bias_cols = _bias_layout()
    bias_idx = {c[:2] + c[2:3]: i for i, c in enumerate(bias_cols)}

    def bcol(layer, k, start):
        return bias_idx[(layer, k, start)]

    nc = bacc.Bacc(None, target_bir_lowering=False, debug=False, num_devices=1)

    xT = nc.dram_tensor("xT", [BASE, R], f32, kind="ExternalInput").ap()
    outT = nc.dram_tensor("outT", [BASE, R], f32, kind="ExternalOutput").ap()
    winT = [
        nc.dram_tensor(f"winT{k}", [SIZES[k], BASE], f32, kind="ExternalInput").ap()
        for k in range(5)
    ]
    woutT = [
        nc.dram_tensor(f"woutT{k}", [BASE, SIZES[k]], f32, kind="ExternalInput").ap()
        for k in range(5)
    ]
    we1T = nc.dram_tensor("we1T", [BASE, H1], f32, kind="ExternalInput").ap()
    we2T = nc.dram_tensor("we2T", [H1, H2], f32, kind="ExternalInput").ap()
    we3T = nc.dram_tensor("we3T", [H2, LAT], f32, kind="ExternalInput").ap()
    wd1T = nc.dram_tensor("wd1T", [LAT, H2], f32, kind="ExternalInput").ap()
    wd2T = nc.dram_tensor("wd2T", [H2, H1], f32, kind="ExternalInput").ap()
    wd3T = nc.dram_tensor("wd3T", [H1, BASE], f32, kind="ExternalInput").ap()
    biasD = nc.dram_tensor("biases", [128, len(bias_cols)], f32,
                           kind="ExternalInput").ap()

    with tile.TileContext(nc) as tc:
        with (
            tc.tile_pool(name="wp", bufs=1) as wp,
            tc.tile_pool(name="ap", bufs=ACT_BUFS) as apool,
            tc.tile_pool(name="pp", bufs=8, space="PSUM") as pp,
        ):
            bias_t = wp.tile([128, len(bias_cols)], f32, tag="bias")
            nc.sync.dma_start(bias_t[:], biasD[:])

            def load_w(dram, n_rows, n_cols, tagbase):
                ts = []
                for i, (ks, kp) in enumerate(_tiles(n_rows)):
                    t = wp.tile([kp, n_cols], f32r, tag=f"{tagbase}_{i}")
                    nc.sync.dma_start(t[:], dram[ks:ks + kp, :].bitcast(f32r))
                    ts.append(t)
                return ts

            win_t = {}
            wout_t = {}
            for k in range(5):
                if c_ks[k] > 0:
                    win_t[k] = load_w(winT[k], SIZES[k], BASE, f"win{k}")
                    wout_t[k] = load_w(woutT[k], BASE, SIZES[k], f"wout{k}")
            we1_t = load_w(we1T, BASE, H1, "we1")
            we2_t = load_w(we2T, H1, H2, "we2")
            we3_t = load_w(we3T, H2, LAT, "we3")
            wd1_t = load_w(wd1T, LAT, H2, "wd1")
            wd2_t = load_w(wd2T, H2, H1, "wd2")
            wd3_t = load_w(wd3T, H1, BASE, "wd3")

            def evac(psum, mp, cn, bias_j, relu, eng, out_dt):
                o = apool.tile([mp, cn], out_dt, tag="act")
                b = bias_t[:mp, bias_j:bias_j + 1]
                if eng == "act":
                    nc.scalar.activation(
                        o[:], psum[:], AF.Relu if relu else AF.Identity, bias=b
                    )
                else:
                    if relu:
                        nc.vector.tensor_scalar(
                            o[:], psum[:], b, 0.0, ALU.add, ALU.max
                        )
                    else:
                        nc.vector.tensor_scalar_add(o[:], psum[:], b)
                return o

            def layer(in_tiles, w_tiles, n_in, n_out, bias_layer, bias_k,
                      relu, eng, cn, out_dt=f32r):
                outs = []
                ktl = _tiles(n_in)
                for (js, jp) in _tiles(n_out):
                    psum = pp.tile([jp, cn], f32, tag="ps")
                    last = len(ktl) - 1
                    for i, (ks, kp) in enumerate(ktl):
                        nc.tensor.matmul(
                            psum[:], w_tiles[i][:, js:js + jp], in_tiles[i][:],
                            start=(i == 0), stop=(i == last),
                        )
                    outs.append(
                        evac(psum, jp, cn, bcol(bias_layer, bias_k, js),
                             relu, eng, out_dt)
                    )
                return outs

            off = 0
            for k in range(5):
                c_k = c_ks[k]
                if c_k <= 0:
                    continue
                s_k = SIZES[k]
                for (c0, cn) in _chunks(c_k):
                    g0 = off + c0
                    # load x^T K-tiles restricted to s_k features
                    xts = []
                    for (ks, kp) in _tiles(s_k):
                        t = apool.tile([kp, cn], f32r, tag="act")
                        nc.sync.dma_start(
                            t[:], xT[ks:ks + kp, g0:g0 + cn].bitcast(f32r)
                        )
                        xts.append(t)
                    e = layer(xts, win_t[k], s_k, BASE, "exp", k,
                              False, "dve", cn)
                    h = layer(e, we1_t, BASE, H1, "L1", 0, True, "act", cn)
                    h = layer(h, we2_t, H1, H2, "L2", 0, True, "act", cn)
                    lat = layer(h, we3_t, H2, LAT, "L3", 0, False, "dve", cn)
                    h = layer(lat, wd1_t, LAT, H2, "D1", 0, True, "act", cn)
                    h = layer(h, wd2_t, H2, H1, "D2", 0, True, "act", cn)
                    dec = layer(h, wd3_t, H1, BASE, "D3", 0, False, "dve", cn)
                    # contract: out rows restricted to s_k
                    ktl = _tiles(BASE)
                    for (os_, op) in _tiles(s_k):
                        psum = pp.tile([op, cn], f32, tag="ps")
                        for i, (ms, mp) in enumerate(ktl):
                            nc.tensor.matmul(
                                psum[:], wout_t[k][i][:, os_:os_ + op], dec[i][:],
                                start=(i == 0), stop=(i == len(ktl) - 1),
                            )
                        ot = evac(psum, op, cn, bcol("out", k, os_),
                                  False, "act", f32)
                        nc.sync.dma_start(
                            outT[os_:os_ + op, g0:g0 + cn], ot[:]
                        )
                off += c_k

    nc.compile()
    return nc


def kernel(**inputs):
    global _last_exec_ns
    from concourse.bass_utils import run_bass_kernel_spmd

    x = np.asarray(inputs["x"], dtype=np.float32)
    seq = np.asarray(inputs["seq_lengths"]).astype(np.int64)
    B = x.shape[0]

    Win = np.asarray(inputs["Win"], dtype=np.float32)
    bin_ = np.asarray(inputs["bin_"], dtype=np.float32)
    Wout = np.asarray(inputs["Wout"], dtype=np.float32)
    bout = np.asarray(inputs["bout"], dtype=np.float32)
    We1 = np.asarray(inputs["We1"], dtype=np.float32)
    be1 = np.asarray(inputs["be1"], dtype=np.float32)
    We2 = np.asarray(inputs["We2"], dtype=np.float32)
    be2 = np.asarray(inputs["be2"], dtype=np.float32)
    We3 = np.asarray(inputs["We3"], dtype=np.float32)
    be3 = np.asarray(inputs["be3"], dtype=np.float32)
    Wd1 = np.asarray(inputs["Wd1"], dtype=np.float32)
    bd1 = np.asarray(inputs["bd1"], dtype=np.float32)
    Wd2 = np.asarray(inputs["Wd2"], dtype=np.float32)
    bd2 = np.asarray(inputs["bd2"], dtype=np.float32)
    Wd3 = np.asarray(inputs["Wd3"], dtype=np.float32)
    bd3 = np.asarray(inputs["bd3"], dtype=np.float32)

    # ---- bucket rows by size ----
    idx = [np.nonzero(seq == s)[0] for s in SIZES]
    n_ks = [len(i) for i in idx]
    # even-rounded per-core counts (float32r needs even moving dims)
    c_ks = tuple(2 * (-(-n // (2 * N_CORES))) if n > 0 else 0 for n in n_ks)
    R = sum(c_ks)
    offs = np.cumsum([0] + list(c_ks))[:-1]

    out = np.zeros((B, BASE), dtype=np.float32)
    if R == 0:
        return out

    # ---- per-core inputs ----
    shared = {}
    for k in range(5):
        s = SIZES[k]
        shared[f"winT{k}"] = np.ascontiguousarray(Win[k].T[:s, :])
        shared[f"woutT{k}"] = np.ascontiguousarray(Wout[k].T[:, :s])
    shared["we1T"] = np.ascontiguousarray(We1.T)
    shared["we2T"] = np.ascontiguousarray(We2.T)
    shared["we3T"] = np.ascontiguousarray(We3.T)
    shared["wd1T"] = np.ascontiguousarray(Wd1.T)
    shared["wd2T"] = np.ascontiguousarray(Wd2.T)
    shared["wd3T"] = np.ascontiguousarray(Wd3.T)

    bias_cols = _bias_layout()
    bp = np.zeros((128, len(bias_cols)), dtype=np.float32)
    vecs = {"L1": be1, "L2": be2, "L3": be3, "D1": bd1, "D2": bd2, "D3": bd3}
    for j, col in enumerate(bias_cols):
        layer, k, start, width = col
        if layer == "exp":
            v = bin_[k][start:start + width]
        elif layer == "out":
            v = bout[k][start:start + width]
        else:
            v = vecs[layer][start:start + width]
        bp[: len(v), j] = v
    shared["biases"] = bp

    in_maps = []
    core_rows = []  # per core: list of (k, rows_valid, local_col_start)
    for m in range(N_CORES):
        Xc = np.zeros((R, BASE), dtype=np.float32)
        rows_info = []
        for k in range(5):
            if c_ks[k] == 0:
                continue
            lo = m * c_ks[k]
            rows = idx[k][lo:lo + c_ks[k]]
            if len(rows):
                Xc[offs[k]:offs[k] + len(rows)] = x[rows]
            rows_info.append((k, rows, offs[k]))
        in_maps.append({"xT": np.ascontiguousarray(Xc.T), **shared})
        core_rows.append(rows_info)

    # ---- build / fetch program ----
    key = (c_ks, R)
    if key not in _prog_cache:
        _prog_cache[key] = _build_program(c_ks, R)
    nc = _prog_cache[key]

    trace = bool(os.environ.get("BASS_TRACE"))
    res = run_bass_kernel_spmd(nc, in_maps, list(range(N_CORES)), trace=trace)
    _last_exec_ns = res.exec_time_ns

    # ---- gather / unsort ----
    for m in range(N_CORES):
        oT = res.results[m]["outT"]
        for (k, rows, o) in core_rows[m]:
            if len(rows):
                out[rows] = oT[:, o:o + len(rows)].T
    return out


# revision 5
# speedup vs baseline: 1.3224x; 1.3224x over previous
"""Trainium2 Bass kernel for nn_FCAutoEncoder (ragged_sequence).

Strategy:
  * Host: bucket rows by seq_length (5 sizes), split each bucket evenly
    across 8 cores (pure data parallel), transpose to feature-major
    [1008, R] per core so activations live as [feat_part, batch_free].
  * Device (per core, identical SPMD program): per bucket k, per column
    chunk (<=512): expand with Win[k] restricted to its true s_k input
    features, shared 1008-512-256-128-256-512-1008 MLP, contract with
    Wout[k] restricted to s_k output features.  All matmuls run as
    float32r (full-rate ~TF32 path, fp32 PSUM accumulation).  PSUM is
    evacuated with fused bias(+ReLU) on ScalarE/VectorE.
  * Host: transpose back, scatter rows to original order; rows beyond
    s_k and rows with unknown lengths are zero.
"""
import math
import os
import sys

sys.path.insert(0, "/opt/trn_rl_repo")

import numpy as np

SIZES = (36, 72, 144, 288, 1008)
BASE = 1008
H1, H2, LAT = 512, 256, 128
N_CORES = 8
MAX_CHUNK = 512
ACT_BUFS = 20

_last_exec_ns = None
_prog_cache = {}


def _tiles(n, t=128):
    return [(s, min(t, n - s)) for s in range(0, n, t)]


def _chunks(c, maxn=MAX_CHUNK):
    """Split c (even) into even-sized chunks <= maxn.

    float32r matmuls require an even moving dim, so every chunk is even.
    """
    if c <= 0:
        return []
    assert c % 2 == 0
    half = c // 2
    n = (c + maxn - 1) // maxn
    base, rem = divmod(half, n)
    out, off = [], 0
    for i in range(n):
        sz = 2 * (base + (1 if i < rem else 0))
        out.append((off, sz))
        off += sz
    return out


def _bias_layout():
    """Fixed column order of the packed [128, NB] bias tensor."""
    cols = []
    for k in range(5):
        for (ms, mp) in _tiles(BASE):
            cols.append(("exp", k, ms, mp))
    for (js, jp) in _tiles(H1):
        cols.append(("L1", 0, js, jp))
    for (js, jp) in _tiles(H2):
        cols.append(("L2", 0, js, jp))
    for (js, jp) in _tiles(LAT):
        cols.append(("L3", 0, js, jp))
    for (js, jp) in _tiles(H2):
        cols.append(("D1", 0, js, jp))
    for (js, jp) in _tiles(H1):
        cols.append(("D2", 0, js, jp))
    for (ms, mp) in _tiles(BASE):
        cols.append(("D3", 0, ms, mp))
    for k in range(5):
        for (os_, op) in _tiles(SIZES[k]):
            cols.append(("out", k, os_, op))
    return cols


def _build_program(c_ks, R):
    import concourse.bacc as bacc
    import concourse.mybir as mybir
    from concourse import tile

    f32 = mybir.dt.float32
    f32r = mybir.dt.float32r
    AF = mybir.ActivationFunctionType
    ALU = mybir.AluOpType

    bias_cols = _bias_layout()
    bias_idx = {c[:2] + c[2:3]: i for i, c in enumerate(bias_cols)}

    def bcol(layer, k, start):
        return bias_idx[(layer, k, start)]

    nc = bacc.Bacc(None, target_bir_lowering=False, debug=False, num_devices=1)

    xT = nc.dram_tensor("xT", [BASE, R], f32, kind="ExternalInput").ap()
    outT = nc.dram_tensor("outT", [BASE, R], f32, kind="ExternalOutput").ap()
    winT = [
        nc.dram_tensor(f"winT{k}", [SIZES[k], BASE], f32, kind="ExternalInput").ap()
        for k in range(5)
    ]
    woutT = [
        nc.dram_tensor(f"woutT{k}", [BASE, SIZES[k]], f32, kind="ExternalInput").ap()
        for k in range(5)
    ]
    we1T = nc.dram_tensor("we1T", [BASE, H1], f32, kind="ExternalInput").ap()
    we2T = nc.dram_tensor("we2T", [H1, H2], f32, kind="ExternalInput").ap()
    we3T = nc.dram_tensor("we3T", [H2, LAT], f32, kind="ExternalInput").ap()
    wd1T = nc.dram_tensor("wd1T", [LAT, H2], f32, kind="ExternalInput").ap()
    wd2T = nc.dram_tensor("wd2T", [H2, H1], f32, kind="ExternalInput").ap()
    wd3T = nc.dram_tensor("wd3T", [H1, BASE], f32, kind="ExternalInput").ap()
    biasD = nc.dram_tensor("biases", [128, len(bias_cols)], f32,
                           kind="ExternalInput").ap()

    with tile.TileContext(nc) as tc:
        with (
            tc.tile_pool(name="wp", bufs=1) as wp,
            tc.tile_pool(name="ap", bufs=ACT_BUFS) as apool,
            tc.tile_pool(name="pp", bufs=8, space="PSUM") as pp,
        ):
            bias_t = wp.tile([128, len(bias_cols)], f32, tag="bias")
            nc.sync.dma_start(bias_t[:], biasD[:])

            def load_w(dram, n_rows, n_cols, tagbase):
                ts = []
                for i, (ks, kp) in enumerate(_tiles(n_rows)):
                    t = wp.tile([kp, n_cols], f32r, tag=f"{tagbase}_{i}")
                    nc.sync.dma_start(t[:], dram[ks:ks + kp, :].bitcast(f32r))
                    ts.append(t)
                return ts

            # weight tiles are loaded lazily, in first-use order, so the
            # first expand's matmuls aren't queued behind ~18MB of DMA
            win_t = {}
            wout_t = {}
            mlp_t = {}

            def mlp_weights():
                if not mlp_t:
                    mlp_t["we1"] = load_w(we1T, BASE, H1, "we1")
                    mlp_t["we2"] = load_w(we2T, H1, H2, "we2")
                    mlp_t["we3"] = load_w(we3T, H2, LAT, "we3")
                    mlp_t["wd1"] = load_w(wd1T, LAT, H2, "wd1")
                    mlp_t["wd2"] = load_w(wd2T, H2, H1, "wd2")
                    mlp_t["wd3"] = load_w(wd3T, H1, BASE, "wd3")
                return mlp_t

            def evac(psum, mp, cn, bias_j, relu, eng, out_dt):
                o = apool.tile([mp, cn], out_dt, tag="act")
                b = bias_t[:mp, bias_j:bias_j + 1]
                if eng == "act":
                    nc.scalar.activation(
                        o[:], psum[:], AF.Relu if relu else AF.Identity, bias=b
                    )
                else:
                    if relu:
                        nc.vector.tensor_scalar(
                            o[:], psum[:], b, 0.0, ALU.add, ALU.max
                        )
                    else:
                        nc.vector.tensor_scalar_add(o[:], psum[:], b)
                return o

            def layer(in_tiles, w_tiles, n_in, n_out, bias_layer, bias_k,
                      relu, eng, cn, out_dt=f32r):
                outs = []
                ktl = _tiles(n_in)
                for (js, jp) in _tiles(n_out):
                    psum = pp.tile([jp, cn], f32, tag="ps")
                    last = len(ktl) - 1
                    for i, (ks, kp) in enumerate(ktl):
                        nc.tensor.matmul(
                            psum[:], w_tiles[i][:, js:js + jp], in_tiles[i][:],
                            start=(i == 0), stop=(i == last),
                        )
                    outs.append(
                        evac(psum, jp, cn, bcol(bias_layer, bias_k, js),
                             relu, eng, out_dt)
                    )
                return outs

            off = 0
            for k in range(5):
                c_k = c_ks[k]
                if c_k <= 0:
                    continue
                s_k = SIZES[k]
                if k not in win_t:
                    win_t[k] = load_w(winT[k], s_k, BASE, f"win{k}")
                for (c0, cn) in _chunks(c_k):
                    g0 = off + c0
                    # load x^T K-tiles restricted to s_k features
                    xts = []
                    for (ks, kp) in _tiles(s_k):
                        t = apool.tile([kp, cn], f32r, tag="act")
                        nc.sync.dma_start(
                            t[:], xT[ks:ks + kp, g0:g0 + cn].bitcast(f32r)
                        )
                        xts.append(t)
                    e = layer(xts, win_t[k], s_k, BASE, "exp", k,
                              False, "dve", cn)
                    w = mlp_weights()
                    h = layer(e, w["we1"], BASE, H1, "L1", 0, True, "act", cn)
                    h = layer(h, w["we2"], H1, H2, "L2", 0, True, "act", cn)
                    lat = layer(h, w["we3"], H2, LAT, "L3", 0, False, "dve", cn)
                    h = layer(lat, w["wd1"], LAT, H2, "D1", 0, True, "act", cn)
                    h = layer(h, w["wd2"], H2, H1, "D2", 0, True, "act", cn)
                    dec = layer(h, w["wd3"], H1, BASE, "D3", 0, False, "dve", cn)
                    if k not in wout_t:
                        wout_t[k] = load_w(woutT[k], BASE, s_k, f"wout{k}")
                    # contract: out rows restricted to s_k
                    ktl = _tiles(BASE)
                    for (os_, op) in _tiles(s_k):
                        psum = pp.tile([op, cn], f32, tag="ps")
                        for i, (ms, mp) in enumerate(ktl):
                            nc.tensor.matmul(
                                psum[:], wout_t[k][i][:, os_:os_ + op], dec[i][:],
                                start=(i == 0), stop=(i == len(ktl) - 1),
                            )
                        ot = evac(psum, op, cn, bcol("out", k, os_),
                                  False, "act", f32)
                        nc.sync.dma_start(
                            outT[os_:os_ + op, g0:g0 + cn], ot[:]
                        )
                off += c_k

    nc.compile()
    return nc


def kernel(**inputs):
    global _last_exec_ns
    from concourse.bass_utils import run_bass_kernel_spmd

    x = np.asarray(inputs["x"], dtype=np.float32)
    seq = np.asarray(inputs["seq_lengths"]).astype(np.int64)
    B = x.shape[0]

    Win = np.asarray(inputs["Win"], dtype=np.float32)
    bin_ = np.asarray(inputs["bin_"], dtype=np.float32)
    Wout = np.asarray(inputs["Wout"], dtype=np.float32)
    bout = np.asarray(inputs["bout"], dtype=np.float32)
    We1 = np.asarray(inputs["We1"], dtype=np.float32)
    be1 = np.asarray(inputs["be1"], dtype=np.float32)
    We2 = np.asarray(inputs["We2"], dtype=np.float32)
    be2 = np.asarray(inputs["be2"], dtype=np.float32)
    We3 = np.asarray(inputs["We3"], dtype=np.float32)
    be3 = np.asarray(inputs["be3"], dtype=np.float32)
    Wd1 = np.asarray(inputs["Wd1"], dtype=np.float32)
    bd1 = np.asarray(inputs["bd1"], dtype=np.float32)
    Wd2 = np.asarray(inputs["Wd2"], dtype=np.float32)
    bd2 = np.asarray(inputs["bd2"], dtype=np.float32)
    Wd3 = np.asarray(inputs["Wd3"], dtype=np.float32)
    bd3 = np.asarray(inputs["bd3"], dtype=np.float32)

    # ---- bucket rows by size ----
    idx = [np.nonzero(seq == s)[0] for s in SIZES]
    n_ks = [len(i) for i in idx]
    # even-rounded per-core counts (float32r needs even moving dims)
    c_ks = tuple(2 * (-(-n // (2 * N_CORES))) if n > 0 else 0 for n in n_ks)
    R = sum(c_ks)
    offs = np.cumsum([0] + list(c_ks))[:-1]

    out = np.zeros((B, BASE), dtype=np.float32)
    if R == 0:
        return out

    # ---- per-core inputs ----
    shared = {}
    for k in range(5):
        s = SIZES[k]
        shared[f"winT{k}"] = np.ascontiguousarray(Win[k].T[:s, :])
        shared[f"woutT{k}"] = np.ascontiguousarray(Wout[k].T[:, :s])
    shared["we1T"] = np.ascontiguousarray(We1.T)
    shared["we2T"] = np.ascontiguousarray(We2.T)
    shared["we3T"] = np.ascontiguousarray(We3.T)
    shared["wd1T"] = np.ascontiguousarray(Wd1.T)
    shared["wd2T"] = np.ascontiguousarray(Wd2.T)
    shared["wd3T"] = np.ascontiguousarray(Wd3.T)

    bias_cols = _bias_layout()
    bp = np.zeros((128, len(bias_cols)), dtype=np.float32)
    vecs = {"L1": be1, "L2": be2, "L3": be3, "D1": bd1, "D2": bd2, "D3": bd3}
    for j, col in enumerate(bias_cols):
        layer, k, start, width = col
        if layer == "exp":
            v = bin_[k][start:start + width]
        elif layer == "out":
            v = bout[k][start:start + width]
        else:
            v = vecs[layer][start:start + width]
        bp[: len(v), j] = v
    shared["biases"] = bp

    in_maps = []
    core_rows = []  # per core: list of (k, rows_valid, local_col_start)
    for m in range(N_CORES):
        Xc = np.zeros((R, BASE), dtype=np.float32)
        rows_info = []
        for k in range(5):
            if c_ks[k] == 0:
                continue
            lo = m * c_ks[k]
            rows = idx[k][lo:lo + c_ks[k]]
            if len(rows):
                Xc[offs[k]:offs[k] + len(rows)] = x[rows]
            rows_info.append((k, rows, offs[k]))
        in_maps.append({"xT": np.ascontiguousarray(Xc.T), **shared})
        core_rows.append(rows_info)

    # ---- build / fetch program ----
    key = (c_ks, R)
    if key not in _prog_cache:
        _prog_cache[key] = _build_program(c_ks, R)
    nc = _prog_cache[key]

    trace = bool(os.environ.get("BASS_TRACE"))
    res = run_bass_kernel_spmd(nc, in_maps, list(range(N_CORES)), trace=trace)
    _last_exec_ns = res.exec_time_ns

    # ---- gather / unsort ----
    for m in range(N_CORES):
        oT = res.results[m]["outT"]
        for (k, rows, o) in core_rows[m]:
            if len(rows):
                out[rows] = oT[:, o:o + len(rows)].T
    return out


# revision 8
# speedup vs baseline: 1.4195x; 1.0734x over previous
"""Trainium2 Bass kernel for nn_FCAutoEncoder (ragged_sequence).

Strategy:
  * Host: bucket rows by seq_length (5 sizes), split each bucket evenly
    across 8 cores (pure data parallel), transpose to feature-major
    [1008, R] per core so activations live as [feat_part, batch_free].
  * Device (per core, identical SPMD program): per bucket k, per column
    chunk (<=512): expand with Win[k] restricted to its true s_k input
    features, shared 1008-512-256-128-256-512-1008 MLP, contract with
    Wout[k] restricted to s_k output features.  All matmuls run as
    float32r (full-rate ~TF32 path, fp32 PSUM accumulation).  PSUM is
    evacuated with fused bias(+ReLU) on ScalarE/VectorE.
  * Host: transpose back, scatter rows to original order; rows beyond
    s_k and rows with unknown lengths are zero.
"""
import math
import os
import sys

sys.path.insert(0, "/opt/trn_rl_repo")

import numpy as np

SIZES = (36, 72, 144, 288, 1008)
BASE = 1008
H1, H2, LAT = 512, 256, 128
N_CORES = 8
MAX_CHUNK = 512
ACT_BUFS = 28

_last_exec_ns = None
_prog_cache = {}


def _tiles(n, t=128):
    return [(s, min(t, n - s)) for s in range(0, n, t)]


def _chunks(c, maxn=MAX_CHUNK):
    """Split c (even) into even-sized chunks <= maxn.

    float32r matmuls require an even moving dim, so every chunk is even.
    """
    if c <= 0:
        return []
    assert c % 2 == 0
    half = c // 2
    n = (c + maxn - 1) // maxn
    base, rem = divmod(half, n)
    out, off = [], 0
    for i in range(n):
        sz = 2 * (base + (1 if i < rem else 0))
        out.append((off, sz))
        off += sz
    return out


def _bias_layout():
    """Fixed column order of the packed [128, NB] bias tensor."""
    cols = []
    for k in range(5):
        for (ms, mp) in _tiles(BASE):
            cols.append(("exp", k, ms, mp))
    for (js, jp) in _tiles(H1):
        cols.append(("L1", 0, js, jp))
    for (js, jp) in _tiles(H2):
        cols.append(("L2", 0, js, jp))
    for (js, jp) in _tiles(LAT):
        cols.append(("L3", 0, js, jp))
    for (js, jp) in _tiles(H2):
        cols.append(("D1", 0, js, jp))
    for (js, jp) in _tiles(H1):
        cols.append(("D2", 0, js, jp))
    for (ms, mp) in _tiles(BASE):
        cols.append(("D3", 0, ms, mp))
    for k in range(5):
        for (os_, op) in _tiles(SIZES[k]):
            cols.append(("out", k, os_, op))
    return cols


def _build_program(c_ks, R):
    import concourse.bacc as bacc
    import concourse.mybir as mybir
    from concourse import tile

    f32 = mybir.dt.float32
    f32r = mybir.dt.float32r
    AF = mybir.ActivationFunctionType
    ALU = mybir.AluOpType

    bias_cols = _bias_layout()
    bias_idx = {c[:2] + c[2:3]: i for i, c in enumerate(bias_cols)}

    def bcol(layer, k, start):
        return bias_idx[(layer, k, start)]

    nc = bacc.Bacc(None, target_bir_lowering=False, debug=False, num_devices=1)

    xT = nc.dram_tensor("xT", [BASE, R], f32, kind="ExternalInput").ap()
    outT = nc.dram_tensor("outT", [BASE, R], f32, kind="ExternalOutput").ap()
    winT = [
        nc.dram_tensor(f"winT{k}", [SIZES[k], BASE], f32, kind="ExternalInput").ap()
        for k in range(5)
    ]
    woutT = [
        nc.dram_tensor(f"woutT{k}", [BASE, SIZES[k]], f32, kind="ExternalInput").ap()
        for k in range(5)
    ]
    we1T = nc.dram_tensor("we1T", [BASE, H1], f32, kind="ExternalInput").ap()
    we2T = nc.dram_tensor("we2T", [H1, H2], f32, kind="ExternalInput").ap()
    we3T = nc.dram_tensor("we3T", [H2, LAT], f32, kind="ExternalInput").ap()
    wd1T = nc.dram_tensor("wd1T", [LAT, H2], f32, kind="ExternalInput").ap()
    wd2T = nc.dram_tensor("wd2T", [H2, H1], f32, kind="ExternalInput").ap()
    wd3T = nc.dram_tensor("wd3T", [H1, BASE], f32, kind="ExternalInput").ap()
    biasD = nc.dram_tensor("biases", [128, len(bias_cols)], f32,
                           kind="ExternalInput").ap()

    with tile.TileContext(nc) as tc:
        with (
            tc.tile_pool(name="wp", bufs=1) as wp,
            tc.tile_pool(name="ap", bufs=ACT_BUFS) as apool,
            tc.tile_pool(name="pp", bufs=8, space="PSUM") as pp,
        ):
            bias_t = wp.tile([128, len(bias_cols)], f32, tag="bias")
            nc.sync.dma_start(bias_t[:], biasD[:])

            def load_w(dram, n_rows, n_cols, tagbase):
                ts = []
                for i, (ks, kp) in enumerate(_tiles(n_rows)):
                    t = wp.tile([kp, n_cols], f32r, tag=f"{tagbase}_{i}")
                    nc.sync.dma_start(t[:], dram[ks:ks + kp, :].bitcast(f32r))
                    ts.append(t)
                return ts

            # weight tiles are loaded lazily, in first-use order, so the
            # first expand's matmuls aren't queued behind ~18MB of DMA
            win_t = {}
            wout_t = {}
            mlp_t = {}

            def mlp_weights():
                if not mlp_t:
                    mlp_t["we1"] = load_w(we1T, BASE, H1, "we1")
                    mlp_t["we2"] = load_w(we2T, H1, H2, "we2")
                    mlp_t["we3"] = load_w(we3T, H2, LAT, "we3")
                    mlp_t["wd1"] = load_w(wd1T, LAT, H2, "wd1")
                    mlp_t["wd2"] = load_w(wd2T, H2, H1, "wd2")
                    mlp_t["wd3"] = load_w(wd3T, H1, BASE, "wd3")
                return mlp_t

            def evac(psum, mp, cn, bias_j, relu, eng, out_dt):
                o = apool.tile([mp, cn], out_dt, tag="act")
                b = bias_t[:mp, bias_j:bias_j + 1]
                if eng == "act":
                    nc.scalar.activation(
                        o[:], psum[:], AF.Relu if relu else AF.Identity, bias=b
                    )
                else:
                    if relu:
                        nc.vector.tensor_scalar(
                            o[:], psum[:], b, 0.0, ALU.add, ALU.max
                        )
                    else:
                        nc.vector.tensor_scalar_add(o[:], psum[:], b)
                return o

            def layer(in_tiles, w_tiles, n_in, n_out, bias_layer, bias_k,
                      relu, eng, cn, out_dt=f32r):
                outs = []
                ktl = _tiles(n_in)
                for (js, jp) in _tiles(n_out):
                    psum = pp.tile([jp, cn], f32, tag="ps")
                    last = len(ktl) - 1
                    for i, (ks, kp) in enumerate(ktl):
                        nc.tensor.matmul(
                            psum[:], w_tiles[i][:, js:js + jp], in_tiles[i][:],
                            start=(i == 0), stop=(i == last),
                        )
                    outs.append(
                        evac(psum, jp, cn, bcol(bias_layer, bias_k, js),
                             relu, eng, out_dt)
                    )
                return outs

            def load_x(k, g0, cn):
                xts = []
                for (ks, kp) in _tiles(SIZES[k]):
                    t = apool.tile([kp, cn], f32r, tag="act")
                    nc.sync.dma_start(
                        t[:], xT[ks:ks + kp, g0:g0 + cn].bitcast(f32r)
                    )
                    xts.append(t)
                return xts

            buckets = [k for k in range(5) if c_ks[k] > 0]
            offs = {}
            off = 0
            for k in range(5):
                offs[k] = off
                off += c_ks[k]

            xpre = {}
            for bi, k in enumerate(buckets):
                s_k = SIZES[k]
                if k not in win_t:
                    win_t[k] = load_w(winT[k], s_k, BASE, f"win{k}")
                chunks = _chunks(c_ks[k])
                for ci, (c0, cn) in enumerate(chunks):
                    g0 = offs[k] + c0
                    xts = xpre.pop(k) if (ci == 0 and k in xpre) \
                        else load_x(k, g0, cn)
                    e = layer(xts, win_t[k], s_k, BASE, "exp", k,
                              False, "dve", cn)
                    w = mlp_weights()
                    if k not in wout_t:
                        wout_t[k] = load_w(woutT[k], BASE, s_k, f"wout{k}")
                    if ci == 0 and bi + 1 < len(buckets):
                        # prefetch next bucket: its first x chunk jumps the
                        # queue ahead of its (large) weight loads
                        nk = buckets[bi + 1]
                        ncn = _chunks(c_ks[nk])[0][1]
                        xpre[nk] = load_x(nk, offs[nk], ncn)
                        win_t[nk] = load_w(winT[nk], SIZES[nk], BASE,
                                           f"win{nk}")
                        wout_t[nk] = load_w(woutT[nk], BASE, SIZES[nk],
                                            f"wout{nk}")
                    h = layer(e, w["we1"], BASE, H1, "L1", 0, True, "act", cn)
                    h = layer(h, w["we2"], H1, H2, "L2", 0, True, "act", cn)
                    lat = layer(h, w["we3"], H2, LAT, "L3", 0, False, "dve", cn)
                    h = layer(lat, w["wd1"], LAT, H2, "D1", 0, True, "act", cn)
                    h = layer(h, w["wd2"], H2, H1, "D2", 0, True, "act", cn)
                    dec = layer(h, w["wd3"], H1, BASE, "D3", 0, False, "dve", cn)
                    # contract: out rows restricted to s_k
                    ktl = _tiles(BASE)
                    for (os_, op) in _tiles(s_k):
                        psum = pp.tile([op, cn], f32, tag="ps")
                        for i, (ms, mp) in enumerate(ktl):
                            nc.tensor.matmul(
                                psum[:], wout_t[k][i][:, os_:os_ + op], dec[i][:],
                                start=(i == 0), stop=(i == len(ktl) - 1),
                            )
                        ot = evac(psum, op, cn, bcol("out", k, os_),
                                  False, "act", f32)
                        nc.sync.dma_start(
                            outT[os_:os_ + op, g0:g0 + cn], ot[:]
                        )

    nc.compile()
    return nc


def kernel(**inputs):
    global _last_exec_ns
    from concourse.bass_utils import run_bass_kernel_spmd

    x = np.asarray(inputs["x"], dtype=np.float32)
    seq = np.asarray(inputs["seq_lengths"]).astype(np.int64)
    B = x.shape[0]

    Win = np.asarray(inputs["Win"], dtype=np.float32)
    bin_ = np.asarray(inputs["bin_"], dtype=np.float32)
    Wout = np.asarray(inputs["Wout"], dtype=np.float32)
    bout = np.asarray(inputs["bout"], dtype=np.float32)
    We1 = np.asarray(inputs["We1"], dtype=np.float32)
    be1 = np.asarray(inputs["be1"], dtype=np.float32)
    We2 = np.asarray(inputs["We2"], dtype=np.float32)
    be2 = np.asarray(inputs["be2"], dtype=np.float32)
    We3 = np.asarray(inputs["We3"], dtype=np.float32)
    be3 = np.asarray(inputs["be3"], dtype=np.float32)
    Wd1 = np.asarray(inputs["Wd1"], dtype=np.float32)
    bd1 = np.asarray(inputs["bd1"], dtype=np.float32)
    Wd2 = np.asarray(inputs["Wd2"], dtype=np.float32)
    bd2 = np.asarray(inputs["bd2"], dtype=np.float32)
    Wd3 = np.asarray(inputs["Wd3"], dtype=np.float32)
    bd3 = np.asarray(inputs["bd3"], dtype=np.float32)

    # ---- bucket rows by size ----
    idx = [np.nonzero(seq == s)[0] for s in SIZES]
    n_ks = [len(i) for i in idx]
    # even-rounded per-core counts (float32r needs even moving dims)
    c_ks = tuple(2 * (-(-n // (2 * N_CORES))) if n > 0 else 0 for n in n_ks)
    R = sum(c_ks)
    offs = np.cumsum([0] + list(c_ks))[:-1]

    out = np.zeros((B, BASE), dtype=np.float32)
    if R == 0:
        return out

    # ---- per-core inputs ----
    shared = {}
    for k in range(5):
        s = SIZES[k]
        shared[f"winT{k}"] = np.ascontiguousarray(Win[k].T[:s, :])
        shared[f"woutT{k}"] = np.ascontiguousarray(Wout[k].T[:, :s])
    shared["we1T"] = np.ascontiguousarray(We1.T)
    shared["we2T"] = np.ascontiguousarray(We2.T)
    shared["we3T"] = np.ascontiguousarray(We3.T)
    shared["wd1T"] = np.ascontiguousarray(Wd1.T)
    shared["wd2T"] = np.ascontiguousarray(Wd2.T)
    shared["wd3T"] = np.ascontiguousarray(Wd3.T)

    bias_cols = _bias_layout()
    bp = np.zeros((128, len(bias_cols)), dtype=np.float32)
    vecs = {"L1": be1, "L2": be2, "L3": be3, "D1": bd1, "D2": bd2, "D3": bd3}
    for j, col in enumerate(bias_cols):
        layer, k, start, width = col
        if layer == "exp":
            v = bin_[k][start:start + width]
        elif layer == "out":
            v = bout[k][start:start + width]
        else:
            v = vecs[layer][start:start + width]
        bp[: len(v), j] = v
    shared["biases"] = bp

    in_maps = []
    core_rows = []  # per core: list of (k, rows_valid, local_col_start)
    for m in range(N_CORES):
        Xc = np.zeros((R, BASE), dtype=np.float32)
        rows_info = []
        for k in range(5):
            if c_ks[k] == 0:
                continue
            lo = m * c_ks[k]
            rows = idx[k][lo:lo + c_ks[k]]
            if len(rows):
                Xc[offs[k]:offs[k] + len(rows)] = x[rows]
            rows_info.append((k, rows, offs[k]))
        in_maps.append({"xT": np.ascontiguousarray(Xc.T), **shared})
        core_rows.append(rows_info)

    # ---- build / fetch program ----
    key = (c_ks, R)
    if key not in _prog_cache:
        _prog_cache[key] = _build_program(c_ks, R)
    nc = _prog_cache[key]

    trace = bool(os.environ.get("BASS_TRACE"))
    res = run_bass_kernel_spmd(nc, in_maps, list(range(N_CORES)), trace=trace)
    _last_exec_ns = res.exec_time_ns

    # ---- gather / unsort ----
    for m in range(N_CORES):
        oT = res.results[m]["outT"]
        for (k, rows, o) in core_rows[m]:
            if len(rows):
                out[rows] = oT[:, o:o + len(rows)].T
    return out


# revision 9
# speedup vs baseline: 1.4995x; 1.0563x over previous
"""Trainium2 Bass kernel for nn_FCAutoEncoder (ragged_sequence).

Strategy:
  * Host: bucket rows by seq_length (5 sizes), split each bucket evenly
    across 8 cores (pure data parallel), transpose to feature-major
    [1024, R] per core so activations live as [feat_part, batch_free].
    All feature dims are zero-padded to multiples of 128 so every
    matmul K-tile is a full 128 partitions (partial-K matmuls measure
    ~2.5x slower on HW).
  * Device (per core, identical SPMD program): per bucket k, per column
    chunk (<=512, even): expand with Win[k] restricted to its true s_k
    input features, shared 1008-512-256-128-256-512-1008 MLP, contract
    with Wout[k] restricted to s_k output features.  All matmuls run as
    float32r (full-rate fp32 path, ~2e-4 rel err, fp32 PSUM accum).
    PSUM is evacuated with fused bias(+ReLU) on ScalarE/VectorE.
    Weights stream in one batched DMA per tensor, in first-use order,
    with one-bucket-ahead prefetch so the PE never waits on HBM.
  * Host: transpose back, scatter rows to original order; rows beyond
    s_k and rows with unknown lengths are zero.
"""
import os
import sys

sys.path.insert(0, "/opt/trn_rl_repo")

import numpy as np

SIZES = (36, 72, 144, 288, 1008)
SP = (128, 128, 256, 384, 1024)   # SIZES padded to multiples of 128
BASE = 1008
BASE_P = 1024
H1, H2, LAT = 512, 256, 128
N_CORES = 8
MAX_CHUNK = 512
ACT_BUFS = 26

_last_exec_ns = None
_prog_cache = {}


def _tiles(n, t=128):
    return [(s, min(t, n - s)) for s in range(0, n, t)]


def _chunks(c, maxn=MAX_CHUNK):
    """Split c (even) into even-sized chunks <= maxn.

    float32r matmuls require an even moving dim, so every chunk is even.
    """
    if c <= 0:
        return []
    assert c % 2 == 0
    half = c // 2
    n = (c + maxn - 1) // maxn
    base, rem = divmod(half, n)
    out, off = [], 0
    for i in range(n):
        sz = 2 * (base + (1 if i < rem else 0))
        out.append((off, sz))
        off += sz
    return out


def _bias_layout():
    """Fixed column order of the packed [128, NB] bias tensor."""
    cols = []
    for k in range(5):
        for (ms, mp) in _tiles(BASE_P):
            cols.append(("exp", k, ms, mp))
    for (js, jp) in _tiles(H1):
        cols.append(("L1", 0, js, jp))
    for (js, jp) in _tiles(H2):
        cols.append(("L2", 0, js, jp))
    for (js, jp) in _tiles(LAT):
        cols.append(("L3", 0, js, jp))
    for (js, jp) in _tiles(H2):
        cols.append(("D1", 0, js, jp))
    for (js, jp) in _tiles(H1):
        cols.append(("D2", 0, js, jp))
    for (ms, mp) in _tiles(BASE_P):
        cols.append(("D3", 0, ms, mp))
    for k in range(5):
        for (os_, op) in _tiles(SIZES[k]):
            cols.append(("out", k, os_, op))
    return cols


def _build_program(c_ks, R):
    import concourse.bacc as bacc
    import concourse.mybir as mybir
    from concourse import tile

    f32 = mybir.dt.float32
    f32r = mybir.dt.float32r
    AF = mybir.ActivationFunctionType
    ALU = mybir.AluOpType

    bias_cols = _bias_layout()
    bias_idx = {c[:3]: i for i, c in enumerate(bias_cols)}

    def bcol(layer, k, start):
        return bias_idx[(layer, k, start)]

    nc = bacc.Bacc(None, target_bir_lowering=False, debug=False, num_devices=1)

    xT = nc.dram_tensor("xT", [BASE_P, R], f32, kind="ExternalInput").ap()
    outT = nc.dram_tensor("outT", [BASE, R], f32, kind="ExternalOutput").ap()
    winT = [
        nc.dram_tensor(f"winT{k}", [SP[k], BASE_P], f32, kind="ExternalInput").ap()
        for k in range(5)
    ]
    woutT = [
        nc.dram_tensor(f"woutT{k}", [BASE_P, SIZES[k]], f32,
                       kind="ExternalInput").ap()
        for k in range(5)
    ]
    we1T = nc.dram_tensor("we1T", [BASE_P, H1], f32, kind="ExternalInput").ap()
    we2T = nc.dram_tensor("we2T", [H1, H2], f32, kind="ExternalInput").ap()
    we3T = nc.dram_tensor("we3T", [H2, LAT], f32, kind="ExternalInput").ap()
    wd1T = nc.dram_tensor("wd1T", [LAT, H2], f32, kind="ExternalInput").ap()
    wd2T = nc.dram_tensor("wd2T", [H2, H1], f32, kind="ExternalInput").ap()
    wd3T = nc.dram_tensor("wd3T", [H1, BASE_P], f32, kind="ExternalInput").ap()
    biasD = nc.dram_tensor("biases", [128, len(bias_cols)], f32,
                           kind="ExternalInput").ap()

    with tile.TileContext(nc) as tc:
        with (
            tc.tile_pool(name="wp", bufs=1) as wp,
            tc.tile_pool(name="ap", bufs=ACT_BUFS) as apool,
            tc.tile_pool(name="pp", bufs=8, space="PSUM") as pp,
        ):
            bias_t = wp.tile([128, len(bias_cols)], f32, tag="bias")
            nc.sync.dma_start(bias_t[:], biasD[:])

            def load_w(dram, n_rows, n_cols, tag):
                """One batched DMA: [t*128, C] dram -> [128, t, C] tile."""
                t = n_rows // 128
                tl = wp.tile([128, t, n_cols], f32r, tag=tag)
                nc.sync.dma_start(
                    tl[:],
                    dram.rearrange("(t p) c -> p t c", p=128).bitcast(f32r),
                )
                return tl

            win_t = {}
            wout_t = {}
            mlp_t = {}

            def mlp_weights():
                if not mlp_t:
                    mlp_t["we1"] = load_w(we1T, BASE_P, H1, "we1")
                    mlp_t["we2"] = load_w(we2T, H1, H2, "we2")
                    mlp_t["we3"] = load_w(we3T, H2, LAT, "we3")
                    mlp_t["wd1"] = load_w(wd1T, LAT, H2, "wd1")
                    mlp_t["wd2"] = load_w(wd2T, H2, H1, "wd2")
                    mlp_t["wd3"] = load_w(wd3T, H1, BASE_P, "wd3")
                return mlp_t

            def evac(psum, mp, cn, bias_j, relu, eng, out_dt):
                o = apool.tile([mp, cn], out_dt, tag="act")
                b = bias_t[:mp, bias_j:bias_j + 1]
                if eng == "act":
                    nc.scalar.activation(
                        o[:], psum[:], AF.Relu if relu else AF.Identity, bias=b
                    )
                else:
                    if relu:
                        nc.vector.tensor_scalar(
                            o[:], psum[:], b, 0.0, ALU.add, ALU.max
                        )
                    else:
                        nc.vector.tensor_scalar_add(o[:], psum[:], b)
                return o

            def layer(in_tiles, wtile, n_in, n_out, bias_layer, bias_k,
                      relu, eng, cn, out_dt=f32r):
                outs = []
                nkt = n_in // 128
                for (js, jp) in _tiles(n_out):
                    psum = pp.tile([jp, cn], f32, tag="ps")
                    for i in range(nkt):
                        nc.tensor.matmul(
                            psum[:], wtile[:, i, js:js + jp], in_tiles[i][:],
                            start=(i == 0), stop=(i == nkt - 1),
                        )
                    outs.append(
                        evac(psum, jp, cn, bcol(bias_layer, bias_k, js),
                             relu, eng, out_dt)
                    )
                return outs

            def load_x(k, g0, cn):
                xts = []
                for (ks, kp) in _tiles(SP[k]):
                    t = apool.tile([kp, cn], f32r, tag="act")
                    nc.sync.dma_start(
                        t[:], xT[ks:ks + kp, g0:g0 + cn].bitcast(f32r)
                    )
                    xts.append(t)
                return xts

            buckets = [k for k in range(5) if c_ks[k] > 0]
            offs = {}
            off = 0
            for k in range(5):
                offs[k] = off
                off += c_ks[k]

            xpre = {}
            for bi, k in enumerate(buckets):
                s_k = SIZES[k]
                if k not in win_t:
                    win_t[k] = load_w(winT[k], SP[k], BASE_P, f"win{k}")
                chunks = _chunks(c_ks[k])
                for ci, (c0, cn) in enumerate(chunks):
                    g0 = offs[k] + c0
                    xts = xpre.pop(k) if (ci == 0 and k in xpre) \
                        else load_x(k, g0, cn)
                    e = layer(xts, win_t[k], SP[k], BASE_P, "exp", k,
                              False, "dve", cn)
                    w = mlp_weights()
                    if k not in wout_t:
                        wout_t[k] = load_w(woutT[k], BASE_P, s_k, f"wout{k}")
                    if ci == 0 and bi + 1 < len(buckets):
                        # prefetch next bucket: its first x chunk jumps the
                        # queue ahead of its (large) weight loads
                        nk = buckets[bi + 1]
                        ncn = _chunks(c_ks[nk])[0][1]
                        xpre[nk] = load_x(nk, offs[nk], ncn)
                        win_t[nk] = load_w(winT[nk], SP[nk], BASE_P,
                                           f"win{nk}")
                        wout_t[nk] = load_w(woutT[nk], BASE_P, SIZES[nk],
                                            f"wout{nk}")
                    h = layer(e, w["we1"], BASE_P, H1, "L1", 0, True, "act", cn)
                    h = layer(h, w["we2"], H1, H2, "L2", 0, True, "act", cn)
                    lat = layer(h, w["we3"], H2, LAT, "L3", 0, False, "dve", cn)
                    h = layer(lat, w["wd1"], LAT, H2, "D1", 0, True, "act", cn)
                    h = layer(h, w["wd2"], H2, H1, "D2", 0, True, "act", cn)
                    dec = layer(h, w["wd3"], H1, BASE_P, "D3", 0, False,
                                "dve", cn)
                    # contract: out rows restricted to the true s_k
                    for (os_, op) in _tiles(s_k):
                        psum = pp.tile([op, cn], f32, tag="ps")
                        for i in range(BASE_P // 128):
                            nc.tensor.matmul(
                                psum[:], wout_t[k][:, i, os_:os_ + op],
                                dec[i][:],
                                start=(i == 0), stop=(i == BASE_P // 128 - 1),
                            )
                        ot = evac(psum, op, cn, bcol("out", k, os_),
                                  False, "act", f32)
                        nc.sync.dma_start(
                            outT[os_:os_ + op, g0:g0 + cn], ot[:]
                        )

    nc.compile()
    return nc


def _pad(a, shape):
    out = np.zeros(shape, dtype=np.float32)
    out[tuple(slice(0, s) for s in a.shape)] = a
    return out


def kernel(**inputs):
    global _last_exec_ns
    from concourse.bass_utils import run_bass_kernel_spmd

    x = np.asarray(inputs["x"], dtype=np.float32)
    seq = np.asarray(inputs["seq_lengths"]).astype(np.int64)
    B = x.shape[0]

    Win = np.asarray(inputs["Win"], dtype=np.float32)
    bin_ = np.asarray(inputs["bin_"], dtype=np.float32)
    Wout = np.asarray(inputs["Wout"], dtype=np.float32)
    bout = np.asarray(inputs["bout"], dtype=np.float32)
    We1 = np.asarray(inputs["We1"], dtype=np.float32)
    be1 = np.asarray(inputs["be1"], dtype=np.float32)
    We2 = np.asarray(inputs["We2"], dtype=np.float32)
    be2 = np.asarray(inputs["be2"], dtype=np.float32)
    We3 = np.asarray(inputs["We3"], dtype=np.float32)
    be3 = np.asarray(inputs["be3"], dtype=np.float32)
    Wd1 = np.asarray(inputs["Wd1"], dtype=np.float32)
    bd1 = np.asarray(inputs["bd1"], dtype=np.float32)
    Wd2 = np.asarray(inputs["Wd2"], dtype=np.float32)
    bd2 = np.asarray(inputs["bd2"], dtype=np.float32)
    Wd3 = np.asarray(inputs["Wd3"], dtype=np.float32)
    bd3 = np.asarray(inputs["bd3"], dtype=np.float32)

    # ---- bucket rows by size ----
    idx = [np.nonzero(seq == s)[0] for s in SIZES]
    n_ks = [len(i) for i in idx]
    # even-rounded per-core counts (float32r needs even moving dims)
    c_ks = tuple(2 * (-(-n // (2 * N_CORES))) if n > 0 else 0 for n in n_ks)
    R = sum(c_ks)

    out = np.zeros((B, BASE), dtype=np.float32)
    if R == 0:
        return out

    offs = np.cumsum([0] + list(c_ks))[:-1]

    # ---- shared (replicated) weight inputs, padded to 128-multiples ----
    shared = {}
    for k in range(5):
        s = SIZES[k]
        shared[f"winT{k}"] = _pad(Win[k].T[:s, :], (SP[k], BASE_P))
        shared[f"woutT{k}"] = _pad(Wout[k].T[:, :s], (BASE_P, s))
    shared["we1T"] = _pad(We1.T, (BASE_P, H1))
    shared["we2T"] = np.ascontiguousarray(We2.T)
    shared["we3T"] = np.ascontiguousarray(We3.T)
    shared["wd1T"] = np.ascontiguousarray(Wd1.T)
    shared["wd2T"] = np.ascontiguousarray(Wd2.T)
    shared["wd3T"] = _pad(Wd3.T, (H1, BASE_P))

    bias_cols = _bias_layout()
    bp = np.zeros((128, len(bias_cols)), dtype=np.float32)
    vecs = {"L1": be1, "L2": be2, "L3": be3, "D1": bd1, "D2": bd2, "D3": bd3}
    for j, col in enumerate(bias_cols):
        layer, k, start, width = col
        if layer == "exp":
            v = bin_[k][start:start + width]
        elif layer == "out":
            v = bout[k][start:start + width]
        else:
            v = vecs[layer][start:start + width]
        bp[: len(v), j] = v
    shared["biases"] = bp

    # ---- per-core inputs ----
    in_maps = []
    core_rows = []
    for m in range(N_CORES):
        Xc = np.zeros((R, BASE_P), dtype=np.float32)
        rows_info = []
        for k in range(5):
            if c_ks[k] == 0:
                continue
            lo = m * c_ks[k]
            rows = idx[k][lo:lo + c_ks[k]]
            if len(rows):
                Xc[offs[k]:offs[k] + len(rows), :BASE] = x[rows]
            rows_info.append((k, rows, offs[k]))
        in_maps.append({"xT": np.ascontiguousarray(Xc.T), **shared})
        core_rows.append(rows_info)

    # ---- build / fetch program ----
    key = (c_ks, R)
    if key not in _prog_cache:
        _prog_cache[key] = _build_program(c_ks, R)
    nc = _prog_cache[key]

    trace = bool(os.environ.get("BASS_TRACE"))
    res = run_bass_kernel_spmd(nc, in_maps, list(range(N_CORES)), trace=trace)
    _last_exec_ns = res.exec_time_ns

    # ---- gather / unsort ----
    for m in range(N_CORES):
        oT = res.results[m]["outT"]
        for (k, rows, o) in core_rows[m]:
            if len(rows):
                out[rows] = oT[:, o:o + len(rows)].T
    return out
